# revision 12
# baseline (speedup 1.0000x reference)
"""Trainium2 kernel for nn_HATGNN: hierarchical label<-patch kNN aggregation.

v2: the 832x100000 cdist+top-k runs on 8 NeuronCores, patch-sharded
(12500/core), with a THREE-ENGINE selection pipeline per 128-label chunk:

- Tensor: one compound fp8-DoubleRow matmul per 2048-col window (one
  LDWEIGHTS + 4 bank-slices) -> exact quantized scores in PSUM (score +
  in-window column index packed into each fp32's mantissa, as in v1).
- Windows are processed in pairs: the EVEN window is copied PSUM->SBUF by
  the Scalar engine; the ODD window stays in PSUM.  A hand-authored custom
  DVE op ("HAT_TOP3", 4-uop FSM) then streams BOTH windows at 2 elem/cycle
  (one PSUM port + one SBUF port) and, per 32-pair group, emits top-2 of
  max(a,b) and top-1 of min(a,b) -- landing as consecutive elements in the
  output stream via SUB_DIM_DONE boundary states.  This costs the DVE
  ~1.17us per 2048-col window vs ~2.28us for the v1 Max8 scan.
- The Scalar engine extracts the per-group trios with one strided copy;
  two small Max8s cover each pair's final group, one Max8 covers the
  212-col tail window.
- Host: decode (score, in-window index) from the fp32 values; each paired
  value may come from either window of its pair, so both columns are added
  as candidates; value-prune, rescore exactly, take the true top-9.
"""
import numpy as np
import ml_dtypes

import concourse.bacc as bacc
import concourse.mybir as mybir
from concourse.tile import TileContext
from concourse.bass_utils import run_bass_kernel_spmd
import concourse.dve_ops as dve_ops
from concourse.dve_ops import DveOp
from concourse.dve_spec import Spec, Src0, Src1, maxx
from concourse.dve_spec import AluOp
from concourse.dve_uop import (DveOpSpec, Trigger, OutPath, OutSel, AluInp,
                               DelayInp, UopConfig, UopDpConfig, InpSel)

NCORES = 8
NPER = 12500          # patches per core
WIN = 2048            # window (4 PSUM banks)
NPAIRW = 3            # window pairs per chunk (windows 0..5)
TAIL = NPER - 6 * WIN  # 212
SPLIT1 = 2048
SPLIT2 = 8192
S = 832               # total labels
SL = 896              # padded to 7 x 128
NCHUNK = SL // 128
C = 256
D = 250               # data channels (6 aux)
NPAIR = 32            # pairs per group
NELEM = 1056          # custom-op stream length (33 pages of 32)
PER_WIN = 104         # 32 trios x3 + 8 (dummy-page top-8)
NCAND = 6 * PER_WIN + 8                # + tail Max8 = 632
EPS = 1e-5

F8 = mybir.dt.float8e4
F32 = mybir.dt.float32
PC_PAD = 4320

_CACHE = {}
LAST_RESULT = None

# ---------------------------------------------------------------------------
# custom DVE op: HAT_TOP3
# ---------------------------------------------------------------------------
_D0, _D1, _D2 = AluInp.PREV_DELAY_0, AluInp.PREV_DELAY_1, AluInp.PREV_DELAY_2
_D3, _D4, _D5 = AluInp.PREV_DELAY_3, AluInp.PREV_DELAY_4, AluInp.PREV_DELAY_5
_P, _C = AluInp.PREV_ALU_OUT, AluInp.CURR_ALU_OUT
_CAP = DelayInp.CURR_ALU_OUT   # capture prev block's register (exclusive)
_CAPP = DelayInp.PREV_ALU_OUT  # capture prev block's output (this element)


def _dp(op=AluOp.BYPASS, s0=_P, s1=_P, caps=(), props=(0, 1, 2)):
    d = UopDpConfig()
    d.op = op
    d.alu_src0 = s0
    d.alu_src1 = s1
    d.alu_out_enable = 1
    dl = list(d.delay)
    de = [0] * 7
    for ln in props:
        dl[ln] = DelayInp.PREV_DELAY
        de[ln] = 1
    for ln, src in caps:
        dl[ln] = src
        de[ln] = 1
    d.delay = dl
    d.delay_enable = de
    return d


def _mk_uop(blocks, trigger, next_uop, repeat=0, req=True, outs=()):
    u = UopConfig()
    u.inp = [InpSel.ZERO] * len(u.inp)
    u.inp_enable = [0] * len(u.inp)
    u.enable_input(InpSel.SRC_0, 1)
    u.enable_input(InpSel.SRC_1, 2)
    u.enable_input(InpSel.MAX_NEG, 3)
    u.trigger = trigger
    u.next_uop = next_uop
    u.repeat_count = repeat
    u.require_inp0 = 1 if req else 0
    u.require_inp1 = 1 if req else 0
    u.datapath_config = blocks
    o = dict(u.out)
    oe = dict(u.out_enable)
    for p_ in OutPath:
        oe[p_] = 0
    for p_, sel in outs:
        o[p_] = sel
        oe[p_] = 1
    u.out = o
    u.out_enable = oe
    return u


def _body(mode):
    st1 = mode == "step1"
    st2 = mode == "step2"
    blocks = [
        _dp(AluOp.MAX, _D0, _D1, props=(0, 1, 2)),                 # x
        _dp(AluOp.MIN, _D0, _D1, caps=[(3, _CAPP)], props=(0, 1, 2)),  # y
        _dp(AluOp.BYPASS if st1 else AluOp.MAX, _D3 if st1 else _C, _D3,
            caps=[(0, _CAPP)], props=(2, 3)),                      # m1
        _dp(AluOp.BYPASS, _P, _P, caps=[(1, _CAP)], props=(0, 2, 3)),
        (_dp(AluOp.BYPASS, _D2, _D2, props=(0, 1, 2)) if st1 else
         _dp(AluOp.MIN, _D1, _D3, props=(0, 1, 2))),               # t1
        _dp(AluOp.BYPASS if st1 else AluOp.MAX, _D0 if st1 else _C, _D0,
            caps=[(5, _CAPP)], props=(1, 2)),                      # n1
        _dp(AluOp.BYPASS if st2 else AluOp.MAX, _D5 if st2 else _C, _D5,
            caps=[(4, _CAPP)], props=(1, 2)),                      # m2
        None,
    ]
    if st2:
        blocks[7] = _dp(AluOp.BYPASS, _P, _P, caps=[(5, _CAP)], props=(1, 2, 4))
    else:
        blocks[7] = _dp(AluOp.BYPASS, _P, _P, props=(1, 2, 4, 5))
    return blocks


def _seed_blocks():
    return [
        _dp(), _dp(),
        _dp(AluOp.BYPASS, _D2, _D2),
        _dp(), _dp(),
        _dp(AluOp.BYPASS, _D2, _D2),
        _dp(AluOp.BYPASS, _D2, _D2),
        _dp(),
    ]


def _hat_top3_uops():
    TD, SD, CT, NO = (Trigger.SRC_TENSOR_DONE, Trigger.SUB_DIM_DONE,
                      Trigger.COUNT, Trigger.NONE)
    seed = _mk_uop(_seed_blocks(), (CT, NO, NO), (1, 0, 0), repeat=1, req=False)
    steady = _mk_uop(_body("steady"), (TD, SD, NO), (0, 2, 0),
                     outs=((OutPath.WR0_LO, OutSel.DELAY_4),))
    step1 = _mk_uop(_body("step1"), (TD, SD, CT), (0, 2, 3), repeat=1,
                    outs=((OutPath.WR0_LO, OutSel.DELAY_1),))
    step2 = _mk_uop(_body("step2"), (TD, SD, CT), (0, 2, 1), repeat=1,
                    outs=((OutPath.WR0_LO, OutSel.DELAY_5),))
    return [seed, steady, step1, step2]


def _register_hat_top3():
    name = "HAT_TOP3"
    if name in dve_ops._SUB_OPCODE_FOR_NAME:
        return next(o for o in dve_ops.OPS if o.name == name)
    op = DveOp(name, Spec(body=maxx(Src0, Src1)), subdim=True, uops_sha={})
    uops = _hat_top3_uops()
    object.__setattr__(op, "_hand_uops", uops)

    def compile_(ver, _op=op, _name=name):
        return DveOpSpec(name=_name, opcode=dve_ops.get_dve_sub_opcode(_name),
                         uops=_op._hand_uops, rd1_en=True)
    object.__setattr__(op, "compile", compile_)
    dve_ops.OPS.append(op)
    dve_ops._SUB_OPCODE_FOR_NAME[name] = (dve_ops._CUSTOM_DVE_ROW_BASE
                                          + len(dve_ops.OPS) - 1)
    dve_ops.CUSTOM_DVE_SPECS[name] = op.spec
    return op


HAT_TOP3 = _register_hat_top3()

# ---------------------------------------------------------------------------
# device program
# ---------------------------------------------------------------------------


def _build_nc():
    nc = bacc.Bacc()
    labT = nc.dram_tensor("labT", [128, 2, SL], F8, kind="ExternalInput")
    patA = nc.dram_tensor("patA", [128, 2, SPLIT1], F8, kind="ExternalInput")
    patB = nc.dram_tensor("patB", [128, 2, SPLIT2 - SPLIT1], F8,
                          kind="ExternalInput")
    patC = nc.dram_tensor("patC", [128, 2, NPER - SPLIT2], F8,
                          kind="ExternalInput")
    cand = nc.dram_tensor("cand", [SL, NCAND], F32, kind="ExternalOutput")

    DR = mybir.MatmulPerfMode.DoubleRow

    with TileContext(nc) as tc:
        with tc.tile_pool(name="big", bufs=1) as bigp, \
             tc.tile_pool(name="work", bufs=NCHUNK) as workp, \
             tc.tile_pool(name="ps", bufs=2, space="PSUM") as psp:
            lab_t = bigp.tile([128, 2, SL], F8, tag="lab")
            pA_t = bigp.tile([128, 2, SPLIT1], F8, tag="pA")
            pB_t = bigp.tile([128, 2, SPLIT2 - SPLIT1], F8, tag="pB")
            pC_t = bigp.tile([128, 2, PC_PAD], F8, tag="pC")
            s0_t = bigp.tile([128, NELEM], F32, tag="s0")
            s1_t = bigp.tile([128, NELEM], F32, tag="s1")
            tr0_t = bigp.tile([128, NELEM], F32, tag="tr0")
            tr1_t = bigp.tile([128, NELEM], F32, tag="tr1")
            s_t = [s0_t, s1_t]
            tr_t = [tr0_t, tr1_t]
            nc.sync.dma_start(out=lab_t[:], in_=labT[:])
            nc.sync.dma_start(out=pA_t[:], in_=patA[:])
            nc.sync.dma_start(out=pB_t[:], in_=patB[:])
            nc.sync.dma_start(out=pC_t[:, :, :NPER - SPLIT2], in_=patC[:])
            # one-time: -inf pad so slots [992:1056] pair against nothing
            for i in range(2):
                nc.gpsimd.memset(s_t[i][:, 992:], -3.0e38)

            def dummy_mms(n, dps):
                # HAM-warming filler: garbage matmuls on label data (already
                # resident), results discarded
                for _ in range(n):
                    nc.tensor.matmul(dps[:, :448], lab_t[:, :, 0:128],
                                     lab_t[:, :, 0:448],
                                     start=True, stop=True, perf_mode=DR)

            def rhs_slice(c0, w):
                if c0 < SPLIT1:
                    return pA_t[:, :, c0:c0 + w]
                if c0 < SPLIT2:
                    return pB_t[:, :, c0 - SPLIT1:c0 - SPLIT1 + w]
                return pC_t[:, :, c0 - SPLIT2:c0 - SPLIT2 + w]

            for lc in range(NCHUNK):
                cv = workp.tile([128, NCAND], F32, tag="cv")
                lab = lab_t[:, :, lc * 128:(lc + 1) * 128]
                # tail window first: its Max8 heads the DVE queue, freeing its
                # PSUM buffer early and decoupling the chunk boundary
                ps_t = psp.tile([128, WIN], F32, tag="ps")
                if lc == 0:
                    dummy_mms(24, ps_t)
                nc.tensor.matmul(ps_t[:, :TAIL], lab, rhs_slice(6 * WIN, TAIL),
                                 start=True, stop=True, perf_mode=DR)
                nc.vector.max(out=cv[:, NCAND - 8:], in_=ps_t[:, :TAIL])
                for w in range(6):
                    w0 = w * WIN
                    gidx = (lc * 6 + w) % 2
                    sb = s_t[gidx]
                    tr = tr_t[gidx]
                    ps = psp.tile([128, WIN], F32, tag="ps")
                    for ti in (2, 3, 0, 1):
                        nc.tensor.matmul(ps[:, ti * 512:(ti + 1) * 512], lab,
                                         rhs_slice(w0 + ti * 512, 512),
                                         start=True, stop=True, perf_mode=DR)
                    if lc == 0 and w == 3:
                        dummy_mms(10, ps_t)
                    # copy cols [1056:2048] beside the remaining bank MMs
                    nc.scalar.copy(out=sb[:, 0:480], in_=ps[:, 1056:1536])
                    nc.scalar.copy(out=sb[:, 480:992], in_=ps[:, 1536:2048])
                    # dummy page's columns (1024..1055) via top-8; issued
                    # first so it doesn't extend the PSUM buffer occupancy
                    nc.vector.max(out=cv[:, w * PER_WIN + 96:w * PER_WIN + 104],
                                  in_=ps[:, 1024:1056])
                    nc.vector._custom_dve(
                        HAT_TOP3,
                        out=tr[:, :NELEM].unsqueeze(-1),
                        in0=ps[:, 0:NELEM].rearrange("p (g k) -> p g k", g=33),
                        in1=sb[:, 0:NELEM].rearrange("p (g k) -> p g k", g=33))
                    base = w * PER_WIN
                    # extraction on GpSimd (keeps the Scalar FIFO unblocked)
                    nc.gpsimd.tensor_copy(
                        cv[:, base:base + 96]
                            .rearrange("p (g k) -> p g k", g=32),
                        tr[:, 31:31 + 1024]
                            .rearrange("p (g k) -> p g k", g=32)[:, :, 0:3])
                nc.gpsimd.dma_start(out=cand[lc * 128:(lc + 1) * 128, :],
                                    in_=cv[:])
    nc.finalize()
    _dedupe_ldweights(nc)
    return nc


def _dedupe_ldweights(nc):
    """Drop consecutive LDWEIGHTS with identical weight operands (the PE
    array already holds them).  Only wait/update-free ones are removed, so
    every embedded semaphore survives on the first LDW of each run."""
    for f in nc.m.functions:
        for bb in f.blocks:
            out = []
            last_sig = None
            for inst in bb.instructions:
                tn = type(inst).__name__
                if tn == "InstLdweights":
                    sig = repr(inst.ins[0])
                    if (sig == last_sig and not inst.has_wait()
                            and not inst.has_update()):
                        continue
                    last_sig = sig
                out.append(inst)
            bb.instructions = out


# ---------------------------------------------------------------------------
# host side (quantization identical to v1)
# ---------------------------------------------------------------------------


def _rotation(labels):
    _, _, Vt = np.linalg.svd(labels.astype(np.float64), full_matrices=True)
    return np.ascontiguousarray(Vt.T.astype(np.float32))


def _quantize_inputs(P, labels):
    V = _rotation(labels)
    Pr = P @ V
    Lr = labels @ V
    psq = (P.astype(np.float64) ** 2).sum(1).astype(np.float32)
    ch = np.zeros((C, NCORES, NPER), np.float32)
    pq = np.clip(np.round(2.0 * Pr[:, :D]) * 0.5, -7.5, 7.5)
    ch[:D] = pq.T.reshape(D, NCORES, NPER)
    A = np.round(psq / 32.0)
    B = np.round(psq - 32.0 * A)
    ch[D] = A.reshape(NCORES, NPER)
    ch[D + 1] = B.reshape(NCORES, NPER)
    j = np.arange(NPER) % WIN
    ch[D + 2] = (((j >> 8) & 7) * 2.0 ** -5)[None, :]
    ch[D + 3] = (((j >> 5) & 7) * 2.0 ** -8)[None, :]
    ch[D + 4] = (((j >> 2) & 7) * 2.0 ** -9)[None, :]
    ch[D + 5] = ((j & 3) * 2.0 ** -9)[None, :]
    rhs = np.ascontiguousarray(ch.transpose(1, 0, 2)).reshape(
        NCORES, 128, 2, NPER).astype(ml_dtypes.float8_e4m3)

    lch = np.zeros((C, SL), np.float32)
    lch[:D, :S] = np.clip(np.round(4.0 * Lr[:, :D]), -15, 15).T
    lch[D, :S] = -64.0
    lch[D + 1, :S] = -2.0
    lch[D + 2, :S] = 1.0
    lch[D + 3, :S] = 1.0
    lch[D + 4, :S] = 0.25
    lch[D + 5, :S] = 2.0 ** -4
    lhsT = np.ascontiguousarray(lch).reshape(128, 2, SL).astype(
        ml_dtypes.float8_e4m3)
    return rhs, lhsT


def _run_device(P, labels):
    global LAST_RESULT
    if "nc" not in _CACHE:
        _CACHE["nc"] = _build_nc()
    nc = _CACHE["nc"]

    rhs, lhsT = _quantize_inputs(P, labels)
    in_maps = []
    for c in range(NCORES):
        in_maps.append({
            "labT": lhsT,
            "patA": np.ascontiguousarray(rhs[c, :, :, :SPLIT1]),
            "patB": np.ascontiguousarray(rhs[c, :, :, SPLIT1:SPLIT2]),
            "patC": np.ascontiguousarray(rhs[c, :, :, SPLIT2:]),
        })
    res = run_bass_kernel_spmd(nc, in_maps, core_ids=list(range(NCORES)))
    LAST_RESULT = res
    return np.stack([np.asarray(r["cand"]) for r in res.results])


# slot -> (windowA, windowB) map for decoding (windowB == windowA for
# single-window slots)
def _slot_windows():
    wA = np.empty(NCAND, np.int64)
    for w in range(6):
        wA[w * PER_WIN:(w + 1) * PER_WIN] = w
    wA[NCAND - 8:] = 6
    return wA * WIN


def _decode_candidates(candv):
    """(8, 896, NCAND) fp32 -> global patch ids (8, 896, NCAND)."""
    wA = _slot_windows()
    v = candv.astype(np.float64)
    s = np.floor(v * 2.0) * 0.5
    with np.errstate(invalid="ignore"):
        j = np.nan_to_num((v - s) * 8192.0, nan=0.0, posinf=0.0, neginf=0.0)
    j = np.rint(j).astype(np.int64)
    bad = ~np.isfinite(candv) | (np.abs(candv) > 1e30) | (j < 0) | (j >= WIN)
    colA = j + wA[None, None, :]
    core = (np.arange(NCORES, dtype=np.int64) * NPER)[:, None, None]
    gA = np.where((colA < NPER) & ~bad, colA + core, -1)
    return gA, candv


def _topk_ctx_exact(labels_sl, gid, vals, P, psq, k=9, keep=2048):
    """Union of candidates -> value-prune -> exact rescore -> top-k -> ctx."""
    n = labels_sl.shape[0]
    g = gid[:, :n].transpose(1, 0, 2).reshape(n, -1)          # (n, 8*NCAND)
    v = vals[:, :n].transpose(1, 0, 2).reshape(n, -1).astype(np.float32)
    v = np.where(g >= 0, v, -np.inf)
    if g.shape[1] > keep:
        sel = np.argpartition(-v, keep - 1, axis=1)[:, :keep]
        g = np.take_along_axis(g, sel, axis=1)
    out = np.empty((n, C), np.float32)
    bs = 64
    for i0 in range(0, n, bs):
        i1 = min(i0 + bs, n)
        gb = g[i0:i1]
        gs = np.where(gb >= 0, gb, 0)
        nb = P[gs]                                            # (b, keep, 256)
        d2 = (psq[gs] - 2.0 * np.einsum('nc,nkc->nk', labels_sl[i0:i1], nb,
                                        optimize=True)
              + (labels_sl[i0:i1] ** 2).sum(-1, keepdims=True)
              ).astype(np.float32)
        d2 = np.where(gb >= 0, d2, np.float32(np.inf))
        # dedupe: equal ids keep first occurrence
        order = np.argsort(d2, axis=1, kind="stable")
        idx_sorted = np.take_along_axis(gs, order, axis=1)
        d2s = np.take_along_axis(d2, order, axis=1)
        picked = np.empty((i1 - i0, k), np.int64)
        for r in range(i1 - i0):
            seen = set()
            cnt = 0
            for j_, pid in enumerate(idx_sorted[r]):
                if d2s[r, j_] == np.inf:
                    break
                if pid in seen:
                    continue
                seen.add(pid)
                picked[r, cnt] = pid
                cnt += 1
                if cnt == k:
                    break
            while cnt < k:
                picked[r, cnt] = picked[r, max(cnt - 1, 0)]
                cnt += 1
        out[i0:i1] = P[picked].max(axis=1) - labels_sl[i0:i1]
    return out


def _label_topk_ctx(labels, tbl, k):
    d2 = ((labels * labels).sum(-1, keepdims=True)
          - 2.0 * labels @ tbl.T + (tbl * tbl).sum(-1)[None, :]).astype(np.float32)
    idx = np.argsort(d2, axis=1, kind="stable")[:, :k]
    nbrs = tbl[idx]
    return nbrs.max(axis=1) - labels


def _layer_norm(x, g, b):
    mu = x.mean(-1, keepdims=True)
    var = x.var(-1, keepdims=True)
    return (x - mu) / np.sqrt(var + EPS) * g + b


def kernel(patch_emb, mood_emb, genre_emb, sub_emb,
           Wm_w, Wm_b, Wg_w, Wg_b, Ws_w, Ws_b,
           lnm_g, lnm_b, lng_g, lng_b, lns_g, lns_b):
    P = np.ascontiguousarray(np.asarray(patch_emb, np.float32))
    mood_e = np.asarray(mood_emb, np.float32)
    genre_e = np.asarray(genre_emb, np.float32)
    sub_e = np.asarray(sub_emb, np.float32)
    labels = np.concatenate([mood_e, genre_e, sub_e], 0)

    candv = _run_device(P, labels)
    gid, vals = _decode_candidates(candv)
    psq = (P.astype(np.float64) ** 2).sum(1).astype(np.float32)

    ctx_m = _topk_ctx_exact(mood_e, gid[:, 0:64], candv[:, 0:64], P, psq)
    mood = _layer_norm(mood_e + np.concatenate([mood_e, ctx_m], -1) @ np.asarray(Wm_w)
                       + np.asarray(Wm_b), np.asarray(lnm_g), np.asarray(lnm_b))

    ctx_gp = _topk_ctx_exact(genre_e, gid[:, 64:320], candv[:, 64:320], P, psq)
    ctx_gm = _label_topk_ctx(genre_e, mood.astype(np.float32), 4)
    genre = _layer_norm(genre_e + np.concatenate([genre_e, ctx_gp, ctx_gm], -1)
                        @ np.asarray(Wg_w) + np.asarray(Wg_b),
                        np.asarray(lng_g), np.asarray(lng_b))

    ctx_sp = _topk_ctx_exact(sub_e, gid[:, 320:832], candv[:, 320:832], P, psq)
    ctx_sm = _label_topk_ctx(sub_e, mood.astype(np.float32), 3)
    ctx_sg = _label_topk_ctx(sub_e, genre.astype(np.float32), 4)
    sub = _layer_norm(sub_e + np.concatenate([sub_e, ctx_sp, ctx_sm, ctx_sg], -1)
                      @ np.asarray(Ws_w) + np.asarray(Ws_b),
                      np.asarray(lns_g), np.asarray(lns_b))

    return np.concatenate([mood, genre, sub], 0).astype(np.float32)


# revision 13
# speedup vs baseline: 1.0665x; 1.0665x over previous
"""Trainium2 kernel for nn_HATGNN: hierarchical label<-patch kNN aggregation.

The 99.9%-of-FLOPs part (832x100000 squared-euclidean cdist + top-9
selection) runs on 8 NeuronCores, patch-sharded (12500 rows/core).

Device-side design (per core):
- Scores z2[s,j] ~= 4L.s @ p_j - 2|p_j|^2 are computed with one fp8e4m3 DoubleRow
  matmul per 512-column tile (K=256 channels in a single pass).
  Operands are quantized to coarse integer grids (labels: round(4l) ints,
  patches: 0.5-grid) so every product lands on a 0.5 grid and the PSUM
  accumulation is EXACT (score magnitude < 2^11, grid 2^-13: 24 bits).
- 6 of the 256 channels are repurposed as aux channels folding in
  (a) -2|p_j|^2 (2 channels, error <= 1 scaled) and (b) the column index
  within its 2048-wide selection window, encoded in mantissa bits
  2^-5..2^-13 (4 channels, exact).  A single fp32 score carries both.
- The ONLY selection work is one Max8 per 2048-wide (4 PSUM banks)
  window: 7 windows x 7 label chunks per core.  No FIND_INDEX8 pass, no
  |p|^2 rank-1 matmul, no index DMA.
- The host decodes (score, column) from the fp32 values, merges the
  8-core union (448 candidates/label), rescores candidates EXACTLY and
  takes the true top-9.  Quantization noise (sigma ~4 on a d^2 scale
  where the union-miss margin is ~40) only affects which candidates
  enter the union, not the final ordering.  The 6 dropped data channels
  are aligned with the label matrix's least-energy singular directions.
- The tiny 3-level MLP/LayerNorm pipeline (<=832 rows) runs in numpy.
"""
import numpy as np
import ml_dtypes

import concourse.bacc as bacc
import concourse.mybir as mybir
from concourse.tile import TileContext
from concourse.bass_utils import run_bass_kernel_spmd

NCORES = 8
NPER = 12500          # patches per core (no padding)
TW = 512              # matmul tile width (one PSUM bank)
WIN = 2048            # selection window (4 PSUM banks)
NWIN = 7              # 6 x 2048 + 1 x 212
SPLIT1 = 2048         # patch SBUF tiles: window 0 | tiles 4-12 | tiles 13-24
SPLIT2 = 6656
S = 832               # total labels (64 mood + 256 genre + 512 sub)
SL = 896              # padded to 7 x 128
NCHUNK = SL // 128    # 7 label chunks
C = 256
D = 250               # data channels (6 aux)
NCAND = NWIN * 8      # 56 candidates per label per core
EPS = 1e-5

F8 = mybir.dt.float8e4
F32 = mybir.dt.float32
PC_PAD = 5856         # pC SBUF tile cols (NPER-SPLIT2=5844 padded to %16==0
                      # for the DoubleRow ko-stride; last 12 cols never read)

_CACHE = {}
LAST_RESULT = None    # BassKernelResults of the most recent device run


def _build_nc():
    nc = bacc.Bacc()
    labT = nc.dram_tensor("labT", [128, 2, SL], F8, kind="ExternalInput")
    patA = nc.dram_tensor("patA", [128, 2, SPLIT1], F8, kind="ExternalInput")
    patB = nc.dram_tensor("patB", [128, 2, SPLIT2 - SPLIT1], F8,
                          kind="ExternalInput")
    patC = nc.dram_tensor("patC", [128, 2, NPER - SPLIT2], F8,
                          kind="ExternalInput")
    cand = nc.dram_tensor("cand", [SL, NCAND], F32, kind="ExternalOutput")

    DR = mybir.MatmulPerfMode.DoubleRow

    with TileContext(nc) as tc:
        with tc.tile_pool(name="big", bufs=1) as bigp, \
             tc.tile_pool(name="work", bufs=NCHUNK) as workp, \
             tc.tile_pool(name="ps", bufs=2, space="PSUM") as psp:
            lab_t = bigp.tile([128, 2, SL], F8, tag="lab")
            pA_t = bigp.tile([128, 2, SPLIT1], F8, tag="pA")
            pB_t = bigp.tile([128, 2, SPLIT2 - SPLIT1], F8, tag="pB")
            pC_t = bigp.tile([128, 2, PC_PAD], F8, tag="pC")
            nc.sync.dma_start(out=lab_t[:], in_=labT[:])
            nc.sync.dma_start(out=pA_t[:], in_=patA[:])
            nc.sync.dma_start(out=pB_t[:], in_=patB[:])
            nc.sync.dma_start(out=pC_t[:, :, :NPER - SPLIT2], in_=patC[:])

            warm_ps = None

            def rhs_slice(c0, w):
                if c0 < SPLIT1:
                    return pA_t[:, :, c0:c0 + w]
                if c0 < SPLIT2:
                    return pB_t[:, :, c0 - SPLIT1:c0 - SPLIT1 + w]
                return pC_t[:, :, c0 - SPLIT2:c0 - SPLIT2 + w]

            for lc in range(NCHUNK):
                cv = workp.tile([128, NCAND], F32, tag="cv")
                if lc == 0:
                    # HAM-warming filler during the input-DMA window: garbage
                    # matmuls on label data (already resident), discarded
                    warm_ps = psp.tile([128, WIN], F32, tag="ps")
                    for _ in range(24):
                        nc.tensor.matmul(warm_ps[:, :448], lab_t[:, :, 0:128],
                                         lab_t[:, :, 0:448],
                                         start=True, stop=True,
                                         perf_mode=mybir.MatmulPerfMode.DoubleRow)
                for w in range(NWIN):
                    w0 = w * WIN
                    wlen = min(WIN, NPER - w0)
                    ps = psp.tile([128, WIN], F32, tag="ps")
                    nt = (wlen + TW - 1) // TW
                    for ti in range(nt):
                        c0 = w0 + ti * TW
                        tw = min(TW, NPER - c0)
                        nc.tensor.matmul(
                            ps[:, ti * TW:ti * TW + tw],
                            lab_t[:, :, lc * 128:(lc + 1) * 128],
                            rhs_slice(c0, tw),
                            start=True, stop=True, perf_mode=DR)
                    nc.vector.max(out=cv[:, w * 8:(w + 1) * 8],
                                  in_=ps[:, :wlen])
                nc.gpsimd.dma_start(out=cand[lc * 128:(lc + 1) * 128, :],
                                    in_=cv[:])
    nc.finalize()
    _dedupe_ldweights(nc)
    return nc


def _dedupe_ldweights(nc):
    """Drop consecutive LDWEIGHTS with identical weight operands (the PE
    array already holds them).  Only wait/update-free ones are removed, so
    every embedded semaphore survives on the first LDW of each run."""
    for f in nc.m.functions:
        for bb in f.blocks:
            out = []
            last_sig = None
            for inst in bb.instructions:
                if type(inst).__name__ == "InstLdweights":
                    sig = repr(inst.ins[0])
                    if (sig == last_sig and not inst.has_wait()
                            and not inst.has_update()):
                        continue
                    last_sig = sig
                out.append(inst)
            bb.instructions = out


def _rotation(labels):
    """Right singular basis of the label matrix, so the 6 dropped data
    channels align with the labels' least-energy directions."""
    _, _, Vt = np.linalg.svd(labels.astype(np.float64), full_matrices=True)
    return np.ascontiguousarray(Vt.T.astype(np.float32))  # (256, 256)


def _quantize_inputs(P, labels):
    """Build per-core fp8 operands with |p|^2 + index aux channels.

    All channel values are exact in e4m3 (DoubleRow pairs are
    magnitude-homogeneous so the reduced-precision in-cell pair-sum is
    exact); products land on a 0.5 grid
    (data), grid >= 2 (|p|^2) or 2^-13..2^-5 (index fields), so the fp32
    accumulation is bit-exact and the host can decode score + index.
    """
    V = _rotation(labels)
    Pr = P @ V                                                    # rotated
    Lr = labels @ V
    psq = (P.astype(np.float64) ** 2).sum(1).astype(np.float32)   # (100000,)
    ch = np.zeros((C, NCORES, NPER), np.float32)
    pq = np.clip(np.round(2.0 * Pr[:, :D]) * 0.5, -7.5, 7.5)      # (N, 250)
    ch[:D] = pq.T.reshape(D, NCORES, NPER)
    A = np.round(psq / 32.0)
    B = np.round(psq - 32.0 * A)
    ch[D] = A.reshape(NCORES, NPER)
    ch[D + 1] = B.reshape(NCORES, NPER)
    j = np.arange(NPER) % WIN
    ch[D + 2] = (((j >> 8) & 7) * 2.0 ** -5)[None, :]             # w=1
    ch[D + 3] = (((j >> 5) & 7) * 2.0 ** -8)[None, :]             # w=1
    ch[D + 4] = (((j >> 2) & 7) * 2.0 ** -9)[None, :]             # w=2^-2
    ch[D + 5] = ((j & 3) * 2.0 ** -9)[None, :]                    # w=2^-4
    rhs = np.ascontiguousarray(ch.transpose(1, 0, 2)).reshape(
        NCORES, 128, 2, NPER).astype(ml_dtypes.float8_e4m3)

    lch = np.zeros((C, SL), np.float32)
    lch[:D, :S] = np.clip(np.round(4.0 * Lr[:, :D]), -15, 15).T
    lch[D, :S] = -64.0
    lch[D + 1, :S] = -2.0
    lch[D + 2, :S] = 1.0
    lch[D + 3, :S] = 1.0
    lch[D + 4, :S] = 0.25
    lch[D + 5, :S] = 2.0 ** -4
    lhsT = np.ascontiguousarray(lch).reshape(128, 2, SL).astype(
        ml_dtypes.float8_e4m3)
    return rhs, lhsT


def _run_device(P, labels):
    """Returns candv (8, 896, 56) fp32 (score + encoded in-window index)."""
    global LAST_RESULT
    if "nc" not in _CACHE:
        _CACHE["nc"] = _build_nc()
    nc = _CACHE["nc"]

    rhs, lhsT = _quantize_inputs(P, labels)
    in_maps = []
    for c in range(NCORES):
        in_maps.append({
            "labT": lhsT,
            "patA": np.ascontiguousarray(rhs[c, :, :, :SPLIT1]),
            "patB": np.ascontiguousarray(rhs[c, :, :, SPLIT1:SPLIT2]),
            "patC": np.ascontiguousarray(rhs[c, :, :, SPLIT2:]),
        })
    res = run_bass_kernel_spmd(nc, in_maps, core_ids=list(range(NCORES)))
    LAST_RESULT = res
    return np.stack([np.asarray(r["cand"]) for r in res.results])


def _decode_candidates(candv):
    """(8, 896, 56) fp32 -> global patch ids (8, 896, 56)."""
    v = candv.astype(np.float64)
    s = np.floor(v * 2.0) * 0.5
    j = np.rint((v - s) * 8192.0).astype(np.int64)          # in-window index
    w = (np.arange(NCAND, dtype=np.int64) // 8) * WIN       # window base col
    col = j + w[None, None, :]
    core = (np.arange(NCORES, dtype=np.int64) * NPER)[:, None, None]
    gid = np.where(col < NPER, col + core, -1)
    return gid, s


def _topk_ctx_exact(labels_sl, gid, P, psq, k=9):
    """Union of per-core candidates -> exact rescore -> top-k -> ctx."""
    n = labels_sl.shape[0]
    g = gid[:, :n].transpose(1, 0, 2).reshape(n, -1)        # (n, 448)
    g_safe = np.where(g >= 0, g, 0)
    nb = P[g_safe]                                          # (n, 448, 256)
    d2 = (psq[g_safe] - 2.0 * np.einsum('nc,nkc->nk', labels_sl, nb,
                                        optimize=True)
          + (labels_sl * labels_sl).sum(-1, keepdims=True)).astype(np.float32)
    d2 = np.where(g >= 0, d2, np.float32(np.inf))
    sel = np.argsort(d2, axis=1, kind="stable")[:, :k]
    idx9 = np.take_along_axis(g_safe, sel, axis=1)
    nbrs = P[idx9]
    return nbrs.max(axis=1) - labels_sl


def _label_topk_ctx(labels, tbl, k):
    """Small exact label<-label aggregation (matches reference ordering)."""
    d2 = ((labels * labels).sum(-1, keepdims=True)
          - 2.0 * labels @ tbl.T + (tbl * tbl).sum(-1)[None, :]).astype(np.float32)
    idx = np.argsort(d2, axis=1, kind="stable")[:, :k]
    nbrs = tbl[idx]
    return nbrs.max(axis=1) - labels


def _layer_norm(x, g, b):
    mu = x.mean(-1, keepdims=True)
    var = x.var(-1, keepdims=True)
    return (x - mu) / np.sqrt(var + EPS) * g + b


def kernel(patch_emb, mood_emb, genre_emb, sub_emb,
           Wm_w, Wm_b, Wg_w, Wg_b, Ws_w, Ws_b,
           lnm_g, lnm_b, lng_g, lng_b, lns_g, lns_b):
    P = np.ascontiguousarray(np.asarray(patch_emb, np.float32))
    mood_e = np.asarray(mood_emb, np.float32)
    genre_e = np.asarray(genre_emb, np.float32)
    sub_e = np.asarray(sub_emb, np.float32)
    labels = np.concatenate([mood_e, genre_e, sub_e], 0)

    candv = _run_device(P, labels)
    gid, _ = _decode_candidates(candv)
    psq = (P.astype(np.float64) ** 2).sum(1).astype(np.float32)

    ctx_m = _topk_ctx_exact(mood_e, gid[:, 0:64], P, psq)
    mood = _layer_norm(mood_e + np.concatenate([mood_e, ctx_m], -1) @ np.asarray(Wm_w)
                       + np.asarray(Wm_b), np.asarray(lnm_g), np.asarray(lnm_b))

    ctx_gp = _topk_ctx_exact(genre_e, gid[:, 64:320], P, psq)
    ctx_gm = _label_topk_ctx(genre_e, mood.astype(np.float32), 4)
    genre = _layer_norm(genre_e + np.concatenate([genre_e, ctx_gp, ctx_gm], -1)
                        @ np.asarray(Wg_w) + np.asarray(Wg_b),
                        np.asarray(lng_g), np.asarray(lng_b))

    ctx_sp = _topk_ctx_exact(sub_e, gid[:, 320:832], P, psq)
    ctx_sm = _label_topk_ctx(sub_e, mood.astype(np.float32), 3)
    ctx_sg = _label_topk_ctx(sub_e, genre.astype(np.float32), 4)
    sub = _layer_norm(sub_e + np.concatenate([sub_e, ctx_sp, ctx_sm, ctx_sg], -1)
                      @ np.asarray(Ws_w) + np.asarray(Ws_b),
                      np.asarray(lns_g), np.asarray(lns_b))

    return np.concatenate([mood, genre, sub], 0).astype(np.float32)



# revision 14
# speedup vs baseline: 1.0993x; 1.0308x over previous
"""Trainium2 kernel for nn_HATGNN: hierarchical label<-patch kNN aggregation.

The 99.9%-of-FLOPs part (832x100000 squared-euclidean cdist + top-9
selection) runs on 8 NeuronCores, patch-sharded (12500 rows/core).

Device-side design (per core):
- Scores z2[s,j] ~= 4L.s @ p_j - 2|p_j|^2 are computed with one fp8e4m3 DoubleRow
  matmul per 512-column tile (K=256 channels in a single pass).
  Operands are quantized to coarse integer grids (labels: round(4l) ints,
  patches: 0.5-grid) so every product lands on a 0.5 grid and the PSUM
  accumulation is EXACT (score magnitude < 2^11, grid 2^-13: 24 bits).
- 6 of the 256 channels are repurposed as aux channels folding in
  (a) -2|p_j|^2 (2 channels, error <= 1 scaled) and (b) the column index
  within its 2048-wide selection window, encoded in mantissa bits
  2^-5..2^-13 (4 channels, exact).  A single fp32 score carries both.
- The ONLY selection work is one Max8 per 2048-wide (4 PSUM banks)
  window: 7 windows x 7 label chunks per core.  No FIND_INDEX8 pass, no
  |p|^2 rank-1 matmul, no index DMA.
- The host decodes (score, column) from the fp32 values, merges the
  8-core union (448 candidates/label), rescores candidates EXACTLY and
  takes the true top-9.  Quantization noise (sigma ~4 on a d^2 scale
  where the union-miss margin is ~40) only affects which candidates
  enter the union, not the final ordering.  The 6 dropped data channels
  are aligned with the label matrix's least-energy singular directions.
- The tiny 3-level MLP/LayerNorm pipeline (<=832 rows) runs in numpy.
"""
import numpy as np
import ml_dtypes

import concourse.bacc as bacc
import concourse.mybir as mybir
from concourse.tile import TileContext
from concourse.bass_utils import run_bass_kernel_spmd

NCORES = 8
NPER = 12500          # patches per core (no padding)
TW = 512              # matmul tile width (one PSUM bank)
WIN = 2048            # selection window (4 PSUM banks)
NWIN = 7              # 6 x 2048 + 1 x 212
SPLIT1 = 2048         # patch SBUF tiles: window 0 | tiles 4-12 | tiles 13-24
SPLIT2 = 6656
S = 832               # total labels (64 mood + 256 genre + 512 sub)
SL = 896              # padded to 7 x 128
NCHUNK = SL // 128    # 7 label chunks
C = 256
D = 250               # data channels (6 aux)
NCAND = NWIN * 8      # 56 candidates per label per core
EPS = 1e-5

F8 = mybir.dt.float8e4
F32 = mybir.dt.float32
PC_PAD = 5856         # pC SBUF tile cols (NPER-SPLIT2=5844 padded to %16==0
                      # for the DoubleRow ko-stride; last 12 cols never read)

_CACHE = {}
LAST_RESULT = None    # BassKernelResults of the most recent device run


def _build_nc():
    nc = bacc.Bacc()
    labT = nc.dram_tensor("labT", [128, 2, SL], F8, kind="ExternalInput")
    patA = nc.dram_tensor("patA", [128, 2, SPLIT1], F8, kind="ExternalInput")
    patB = nc.dram_tensor("patB", [128, 2, SPLIT2 - SPLIT1], F8,
                          kind="ExternalInput")
    patC = nc.dram_tensor("patC", [128, 2, NPER - SPLIT2], F8,
                          kind="ExternalInput")
    cand = nc.dram_tensor("cand", [SL, NCAND], F32, kind="ExternalOutput")

    DR = mybir.MatmulPerfMode.DoubleRow

    with TileContext(nc) as tc:
        with tc.tile_pool(name="big", bufs=1) as bigp, \
             tc.tile_pool(name="work", bufs=NCHUNK) as workp, \
             tc.tile_pool(name="ps", bufs=2, space="PSUM") as psp:
            lab_t = bigp.tile([128, 2, SL], F8, tag="lab")
            pA_t = bigp.tile([128, 2, SPLIT1], F8, tag="pA")
            pB_t = bigp.tile([128, 2, SPLIT2 - SPLIT1], F8, tag="pB")
            pC_t = bigp.tile([128, 2, PC_PAD], F8, tag="pC")
            nc.sync.dma_start(out=lab_t[:], in_=labT[:])
            nc.sync.dma_start(out=pA_t[:], in_=patA[:])
            nc.sync.dma_start(out=pB_t[:], in_=patB[:])
            nc.sync.dma_start(out=pC_t[:, :, :NPER - SPLIT2], in_=patC[:])

            def rhs_slice(c0, w):
                if c0 < SPLIT1:
                    return pA_t[:, :, c0:c0 + w]
                if c0 < SPLIT2:
                    return pB_t[:, :, c0 - SPLIT1:c0 - SPLIT1 + w]
                return pC_t[:, :, c0 - SPLIT2:c0 - SPLIT2 + w]

            for lc in range(NCHUNK):
                cv = workp.tile([128, NCAND], F32, tag="cv")
                for w in range(NWIN):
                    w0 = w * WIN
                    wlen = min(WIN, NPER - w0)
                    ps = psp.tile([128, WIN], F32, tag="ps")
                    nt = (wlen + TW - 1) // TW
                    for ti in range(nt):
                        c0 = w0 + ti * TW
                        tw = min(TW, NPER - c0)
                        nc.tensor.matmul(
                            ps[:, ti * TW:ti * TW + tw],
                            lab_t[:, :, lc * 128:(lc + 1) * 128],
                            rhs_slice(c0, tw),
                            start=True, stop=True, perf_mode=DR)
                    nc.vector.max(out=cv[:, w * 8:(w + 1) * 8],
                                  in_=ps[:, :wlen])
                nc.gpsimd.dma_start(out=cand[lc * 128:(lc + 1) * 128, :],
                                    in_=cv[:])
    nc.finalize()
    return nc


def _rotation(labels):
    """Right singular basis of the label matrix, so the 6 dropped data
    channels align with the labels' least-energy directions."""
    _, _, Vt = np.linalg.svd(labels.astype(np.float64), full_matrices=True)
    return np.ascontiguousarray(Vt.T.astype(np.float32))  # (256, 256)


def _quantize_inputs(P, labels):
    """Build per-core fp8 operands with |p|^2 + index aux channels.

    All channel values are exact in e4m3 (DoubleRow pairs are
    magnitude-homogeneous so the reduced-precision in-cell pair-sum is
    exact); products land on a 0.5 grid
    (data), grid >= 2 (|p|^2) or 2^-13..2^-5 (index fields), so the fp32
    accumulation is bit-exact and the host can decode score + index.
    """
    V = _rotation(labels)
    Pr = P @ V                                                    # rotated
    Lr = labels @ V
    psq = (P.astype(np.float64) ** 2).sum(1).astype(np.float32)   # (100000,)
    ch = np.zeros((C, NCORES, NPER), np.float32)
    pq = np.clip(np.round(2.0 * Pr[:, :D]) * 0.5, -7.5, 7.5)      # (N, 250)
    ch[:D] = pq.T.reshape(D, NCORES, NPER)
    A = np.round(psq / 32.0)
    B = np.round(psq - 32.0 * A)
    ch[D] = A.reshape(NCORES, NPER)
    ch[D + 1] = B.reshape(NCORES, NPER)
    j = np.arange(NPER) % WIN
    ch[D + 2] = (((j >> 8) & 7) * 2.0 ** -5)[None, :]             # w=1
    ch[D + 3] = (((j >> 5) & 7) * 2.0 ** -8)[None, :]             # w=1
    ch[D + 4] = (((j >> 2) & 7) * 2.0 ** -9)[None, :]             # w=2^-2
    ch[D + 5] = ((j & 3) * 2.0 ** -9)[None, :]                    # w=2^-4
    rhs = np.ascontiguousarray(ch.transpose(1, 0, 2)).reshape(
        NCORES, 128, 2, NPER).astype(ml_dtypes.float8_e4m3)

    lch = np.zeros((C, SL), np.float32)
    lch[:D, :S] = np.clip(np.round(4.0 * Lr[:, :D]), -15, 15).T
    lch[D, :S] = -64.0
    lch[D + 1, :S] = -2.0
    lch[D + 2, :S] = 1.0
    lch[D + 3, :S] = 1.0
    lch[D + 4, :S] = 0.25
    lch[D + 5, :S] = 2.0 ** -4
    lhsT = np.ascontiguousarray(lch).reshape(128, 2, SL).astype(
        ml_dtypes.float8_e4m3)
    return rhs, lhsT


def _run_device(P, labels):
    """Returns candv (8, 896, 56) fp32 (score + encoded in-window index)."""
    global LAST_RESULT
    if "nc" not in _CACHE:
        _CACHE["nc"] = _build_nc()
    nc = _CACHE["nc"]

    rhs, lhsT = _quantize_inputs(P, labels)
    in_maps = []
    for c in range(NCORES):
        in_maps.append({
            "labT": lhsT,
            "patA": np.ascontiguousarray(rhs[c, :, :, :SPLIT1]),
            "patB": np.ascontiguousarray(rhs[c, :, :, SPLIT1:SPLIT2]),
            "patC": np.ascontiguousarray(rhs[c, :, :, SPLIT2:]),
        })
    res = run_bass_kernel_spmd(nc, in_maps, core_ids=list(range(NCORES)))
    LAST_RESULT = res
    return np.stack([np.asarray(r["cand"]) for r in res.results])


def _decode_candidates(candv):
    """(8, 896, 56) fp32 -> global patch ids (8, 896, 56)."""
    v = candv.astype(np.float64)
    s = np.floor(v * 2.0) * 0.5
    j = np.rint((v - s) * 8192.0).astype(np.int64)          # in-window index
    w = (np.arange(NCAND, dtype=np.int64) // 8) * WIN       # window base col
    col = j + w[None, None, :]
    core = (np.arange(NCORES, dtype=np.int64) * NPER)[:, None, None]
    gid = np.where(col < NPER, col + core, -1)
    return gid, s


def _topk_ctx_exact(labels_sl, gid, P, psq, k=9):
    """Union of per-core candidates -> exact rescore -> top-k -> ctx."""
    n = labels_sl.shape[0]
    g = gid[:, :n].transpose(1, 0, 2).reshape(n, -1)        # (n, 448)
    g_safe = np.where(g >= 0, g, 0)
    nb = P[g_safe]                                          # (n, 448, 256)
    d2 = (psq[g_safe] - 2.0 * np.einsum('nc,nkc->nk', labels_sl, nb,
                                        optimize=True)
          + (labels_sl * labels_sl).sum(-1, keepdims=True)).astype(np.float32)
    d2 = np.where(g >= 0, d2, np.float32(np.inf))
    sel = np.argsort(d2, axis=1, kind="stable")[:, :k]
    idx9 = np.take_along_axis(g_safe, sel, axis=1)
    nbrs = P[idx9]
    return nbrs.max(axis=1) - labels_sl


def _label_topk_ctx(labels, tbl, k):
    """Small exact label<-label aggregation (matches reference ordering)."""
    d2 = ((labels * labels).sum(-1, keepdims=True)
          - 2.0 * labels @ tbl.T + (tbl * tbl).sum(-1)[None, :]).astype(np.float32)
    idx = np.argsort(d2, axis=1, kind="stable")[:, :k]
    nbrs = tbl[idx]
    return nbrs.max(axis=1) - labels


def _layer_norm(x, g, b):
    mu = x.mean(-1, keepdims=True)
    var = x.var(-1, keepdims=True)
    return (x - mu) / np.sqrt(var + EPS) * g + b


def kernel(patch_emb, mood_emb, genre_emb, sub_emb,
           Wm_w, Wm_b, Wg_w, Wg_b, Ws_w, Ws_b,
           lnm_g, lnm_b, lng_g, lng_b, lns_g, lns_b):
    P = np.ascontiguousarray(np.asarray(patch_emb, np.float32))
    mood_e = np.asarray(mood_emb, np.float32)
    genre_e = np.asarray(genre_emb, np.float32)
    sub_e = np.asarray(sub_emb, np.float32)
    labels = np.concatenate([mood_e, genre_e, sub_e], 0)

    candv = _run_device(P, labels)
    gid, _ = _decode_candidates(candv)
    psq = (P.astype(np.float64) ** 2).sum(1).astype(np.float32)

    ctx_m = _topk_ctx_exact(mood_e, gid[:, 0:64], P, psq)
    mood = _layer_norm(mood_e + np.concatenate([mood_e, ctx_m], -1) @ np.asarray(Wm_w)
                       + np.asarray(Wm_b), np.asarray(lnm_g), np.asarray(lnm_b))

    ctx_gp = _topk_ctx_exact(genre_e, gid[:, 64:320], P, psq)
    ctx_gm = _label_topk_ctx(genre_e, mood.astype(np.float32), 4)
    genre = _layer_norm(genre_e + np.concatenate([genre_e, ctx_gp, ctx_gm], -1)
                        @ np.asarray(Wg_w) + np.asarray(Wg_b),
                        np.asarray(lng_g), np.asarray(lng_b))

    ctx_sp = _topk_ctx_exact(sub_e, gid[:, 320:832], P, psq)
    ctx_sm = _label_topk_ctx(sub_e, mood.astype(np.float32), 3)
    ctx_sg = _label_topk_ctx(sub_e, genre.astype(np.float32), 4)
    sub = _layer_norm(sub_e + np.concatenate([sub_e, ctx_sp, ctx_sm, ctx_sg], -1)
                      @ np.asarray(Ws_w) + np.asarray(Ws_b),
                      np.asarray(lns_g), np.asarray(lns_b))

    return np.concatenate([mood, genre, sub], 0).astype(np.float32)

